# revision 1
# baseline (speedup 1.0000x reference)
"""BallQueryAttention TRN2 kernel (v7).

Math: reference computes softmax over a binary ball mask (d2 <= R^2), then
mask-softmax @ x.  exp of a 0/1 mask takes only values {1, e}, so

  out[i] = (S + (e-1) * sum_{j in ball(i)} x_j) / (N + (e-1) * cnt_i)

with S = colsum(x).  Sharding: rows (i) across 8 cores, x replicated
(all-gather-equivalent: every core receives the full x-derived operands).

Host-side input formatting (all O(N*D): dtype casts, reshapes, row sums
-- the O(N^2 * D) compute stays on device):
  - W   [128, N] fp16: rows [hiT_j(64) | ones(3) | loT_j(61)] -- the Gram
    stationary.  (fp16 hi/lo split; the j-side lo correction is dropped on
    dims 61:64, ~1e-3 d2 error, far under the threshold noise floor.)
  - XW  [128, 65*JT] fp16: [x_hi | 1] per j-tile (pass-C stationary).
  - R1  [128, ROWS] fp16: [hiT_i; zeros3; hiT_i(0:61)] (M1 moving).
  - R2v [67, ROWS] fp16: [loT_i(64); v1; v2; v3] (M2 moving; v = 3-level
    fp16 split of -0.5*sq_i).
  - thrD/biasA [128, JT] fp32: +-(sq_j - R^2)/2 per-partition thresholds.
  - ssum [65, 1] fp32: [colsum(x); N].
  i labeling is transpose-natural (col t*128+p <-> row p*8+t), undone by
  the contiguous-run output DMA.

Device per core (row shard of 1024):
  - Gram: 2 fp16 matmuls per 512-col half into PSUM:
      Gh = W^T @ R1  (hh + lh61; ones rows hit R1's zero rows)
         + W[0:67]^T @ R2v  (hl + sq_i aug rows)
  - mask compare split Vector (is_ge -> {0,2}) / Scalar (Sign -> {-1,1}).
  - pass C: accumulating XW^T @ mask -> OUT2 [65, 1024] PSUM.
  - tail: scale by K1 with SALL bias, PE transpose, reciprocal-divide,
    one contiguous output DMA.
"""

import sys

sys.path.insert(0, "/opt/trn_rl_repo")

import numpy as np

import concourse.bass as bass
import concourse.tile as tile
from concourse import bacc, masks, mybir
from concourse.bass_utils import run_bass_kernel_spmd

F32 = mybir.dt.float32
F16 = mybir.dt.float16
AF = mybir.ActivationFunctionType
OP = mybir.AluOpType

N = 8192
D = 64
NCORES = 8
ROWS = N // NCORES          # 1024 rows per core
JT = N // 128               # 64 j-tiles
IT = ROWS // 128            # 8 i-tiles
R2 = 11.0 * 11.0
K1 = (np.e - 1.0) / 2.0
LAG = 4                     # pass-C lag in half-tiles
NLO = 61                    # j-side lo dims kept


def _setup(nc, tc, pools):
    """Iteration-invariant: identity matrix + act-table warm."""
    const, scratch, gpool, mpool, apool, spool = pools
    IDN = const.tile([65, 65], F32, tag="IDN")
    ONEC = const.tile([128, 1], F16, tag="ONEC")
    nc.vector.memset(ONEC[:], 1.0)
    masks.make_identity(nc, IDN[:])
    dumm = spool.tile([128, 1], F32, tag="dumm")
    nc.scalar.activation(dumm[:], ONEC[:], AF.Sign)
    return dict(IDN=IDN)


def _body(nc, tc, pools, env, wd, xwd, r1d, r2d, thrd, biasd, ssum, outd):
    const, scratch, gpool, mpool, apool, spool = pools
    ts = bass.ts
    IDN = env["IDN"]

    # ---------------- operand tiles + input DMA ----------------
    W = const.tile([128, N], F16, tag="W")
    XW = const.tile([128, 65 * JT], F16, tag="XW")
    R1 = const.tile([128, ROWS], F16, tag="R1")
    R2v = const.tile([67, ROWS], F16, tag="R2v")
    biasA = const.tile([128, JT], F32, tag="biasA")
    thrD = const.tile([128, JT], F32, tag="thrD")

    # small tiles first (mask thresholds + i-side movings), then the big
    # stationaries in tile-order chunks so the main loop starts early
    nc.sync.dma_start(thrD[:], thrd)
    nc.scalar.dma_start(biasA[:], biasd)
    nc.sync.dma_start(R1[:], r1d)
    nc.scalar.dma_start(R2v[:], r2d)
    sallsb = spool.tile([65, 1], F32, tag="sallsb")
    nc.gpsimd.dma_start(sallsb[:], ssum)
    CHUNKS = (4, 12, 16, 16, 16)
    pos = 0
    for ci, w in enumerate(CHUNKS):
        eng = nc.sync if ci % 2 == 0 else nc.scalar
        eng.dma_start(W[:, pos * 128 : (pos + w) * 128],
                      wd[:, pos * 128 : (pos + w) * 128])
        eng.dma_start(XW[:, pos * 65 : (pos + w) * 65],
                      xwd[:, pos * 65 : (pos + w) * 65])
        pos += w

    OUT2 = apool.tile([65, ROWS], F32, tag="OUT2")

    # ---------------- main loop ----------------
    NH = 2 * JT
    mks = {}
    for idx in range(NH + LAG):
        if idx < NH:
            t, h = divmod(idx, 2)
            cs = slice(512 * h, 512 * (h + 1))
            Gh = gpool.tile([128, 512], F32, tag="G")
            nc.tensor.matmul(Gh[:], W[:, ts(t, 128)], R1[:, cs],
                             start=True, stop=False)
            nc.tensor.matmul(Gh[:], W[0:67, ts(t, 128)], R2v[0:67, cs],
                             start=False, stop=True)
            mk = mpool.tile([128, 512], F16, tag="mk")
            if idx % 2 == 0:
                nc.vector.tensor_scalar(mk[:], Gh[:], thrD[:, t : t + 1],
                                        2.0, OP.is_ge, OP.mult)
            else:
                nc.scalar.activation(mk[:], Gh[:], AF.Sign,
                                     bias=biasA[:, t : t + 1])
            mks[idx] = mk
        if idx >= LAG:
            jdx = idx - LAG
            t, h = divmod(jdx, 2)
            cs = slice(512 * h, 512 * (h + 1))
            nc.tensor.matmul(OUT2[:, cs], XW[:, 65 * t : 65 * (t + 1)],
                             mks.pop(jdx)[:],
                             start=(t == 0), stop=(t == JT - 1))

    # ---------------- tail -----------------------------------------
    b1sb = spool.tile([65, 1], F32, tag="b1sb")
    nc.vector.tensor_scalar(b1sb[:], sallsb[:], 1.0 + K1, None, OP.mult)

    # all chunks accumulate into one SBUF tile; ot[p, c*D:..] holds row
    # p*8+c, so one contiguous-run DMA (2KB/partition) writes the output
    ot = spool.tile([128, IT * D], F32, tag="ot")
    for c in range(IT):
        bap = sallsb if c < IT // 2 else b1sb
        pc = spool.tile([65, 128], F32, tag=f"pc{c % 2}")
        if c % 2 == 0:
            nc.vector.tensor_scalar(pc[:], OUT2[:, ts(c, 128)], K1, bap[:],
                                    OP.mult, OP.add)
        else:
            nc.scalar.activation(pc[:], OUT2[:, ts(c, 128)], AF.Identity,
                                 bias=bap[:], scale=K1)
        pt = gpool.tile([128, 65], F32, tag="G")
        nc.tensor.transpose(pt[:], pc[:], IDN[:])
        dinv = spool.tile([128, 1], F32, tag=f"dinv{c % 2}")
        nc.vector.reciprocal(dinv[:], pt[:, D : D + 1])
        if c % 2 == 0:
            nc.vector.tensor_scalar(ot[:, ts(c, D)], pt[:, 0:D], dinv[:],
                                    None, OP.mult)
        else:
            nc.scalar.activation(ot[:, ts(c, D)], pt[:, 0:D], AF.Identity,
                                 scale=dinv[:])
    nc.sync.dma_start(outd.rearrange("(p t) d -> p (t d)", p=128), ot[:])


def build_module(loop_n=1, scope="full"):
    nc = bacc.Bacc("TRN2", target_bir_lowering=False, debug=False,
                   num_devices=NCORES)
    wd = nc.dram_tensor("wd", [128, N], F16, kind="ExternalInput")
    xwd = nc.dram_tensor("xwd", [128, 65 * JT], F16, kind="ExternalInput")
    r1d = nc.dram_tensor("r1d", [128, ROWS], F16, kind="ExternalInput")
    r2d = nc.dram_tensor("r2d", [67, ROWS], F16, kind="ExternalInput")
    thrd = nc.dram_tensor("thrd", [128, JT], F32, kind="ExternalInput")
    biasd = nc.dram_tensor("biasd", [128, JT], F32, kind="ExternalInput")
    ssum_d = nc.dram_tensor("ssum", [65, 1], F32, kind="ExternalInput")
    out_d = nc.dram_tensor("out", [ROWS, D], F32, kind="ExternalOutput")

    with tile.TileContext(nc) as tc:
        with (
            tc.tile_pool(name="const", bufs=1) as const,
            tc.tile_pool(name="scratch", bufs=2) as scratch,
            tc.tile_pool(name="gpool", bufs=6, space="PSUM") as gpool,
            tc.tile_pool(name="mk", bufs=8) as mpool,
            tc.tile_pool(name="acc", bufs=1, space="PSUM") as apool,
            tc.tile_pool(name="small", bufs=3) as spool,
        ):
            pools = (const, scratch, gpool, mpool, apool, spool)
            env = _setup(nc, tc, pools)
            args = (nc, tc, pools, env, wd.ap(), xwd.ap(), r1d.ap(),
                    r2d.ap(), thrd.ap(), biasd.ap(), ssum_d.ap(), out_d.ap())
            if loop_n == 1:
                _body(*args)
            else:
                engs = (mybir.EngineType.PE, mybir.EngineType.DVE,
                        mybir.EngineType.Activation, mybir.EngineType.Pool,
                        mybir.EngineType.SP)
                with tc.For_i(0, loop_n, hint_engines=engs,
                              staggered_reset=True) as _:
                    _body(*args)
    nc.finalize()
    return nc


def prep_inputs(x):
    """Host-side input formatting: dtype casts, transposes, row sums."""
    x = np.ascontiguousarray(np.asarray(x, dtype=np.float32))
    xh = x.astype(np.float16)
    xl = (x - xh.astype(np.float32)).astype(np.float16)

    x3h = xh.reshape(128, JT, D)                # row p*64+t (j side)
    x3l = xl.reshape(128, JT, D)
    W = np.empty((128, N), dtype=np.float16)
    W3 = W.reshape(128, JT, 128)                # [e, t, p]
    W3[0:D] = x3h.transpose(2, 1, 0)
    W3[D : D + 3] = np.float16(1.0)
    W3[D + 3 : 128] = x3l.transpose(2, 1, 0)[0:NLO]

    XW = np.empty((128, JT, 65), dtype=np.float16)
    XW[:, :, 0:D] = x3h
    XW[:, :, D] = np.float16(1.0)
    XW = XW.reshape(128, JT * 65)

    sq = (x.astype(np.float64) ** 2).sum(1).astype(np.float32)
    thrD = ((sq - R2) / 2).reshape(128, JT)
    biasA = np.ascontiguousarray(-thrD)

    ssum = np.concatenate([x.sum(0, dtype=np.float64).astype(np.float32),
                           np.float32([N])]).reshape(65, 1)

    per_core = []
    for c in range(NCORES):
        xih = xh[c * ROWS : (c + 1) * ROWS].reshape(128, IT, D)
        xil = xl[c * ROWS : (c + 1) * ROWS].reshape(128, IT, D)
        R1 = np.zeros((128, ROWS), dtype=np.float16)
        R13 = R1.reshape(128, IT, 128)                      # [e, t, p]
        R13[0:D] = xih.transpose(2, 1, 0)
        R13[D + 3 : 128] = xih.transpose(2, 1, 0)[0:NLO]
        R2s = np.empty((128, IT, 67), dtype=np.float16)     # [p, t, e]
        R2s[:, :, 0:D] = xil
        v = -0.5 * sq[c * ROWS : (c + 1) * ROWS].reshape(128, IT)
        v1 = v.astype(np.float16)
        rv = v - v1.astype(np.float32)
        v2 = rv.astype(np.float16)
        v3 = (rv - v2.astype(np.float32)).astype(np.float16)
        R2s[:, :, D] = v1
        R2s[:, :, D + 1] = v2
        R2s[:, :, D + 2] = v3
        R2v = np.ascontiguousarray(
            R2s.transpose(2, 1, 0).reshape(67, ROWS))
        per_core.append({"wd": W, "xwd": XW, "r1d": np.ascontiguousarray(R1),
                         "r2d": R2v, "thrd": thrD, "biasd": biasA,
                         "ssum": ssum})
    return per_core


_module_cache = {}


def _get_module(loop_n=1):
    if loop_n not in _module_cache:
        _module_cache[loop_n] = build_module(loop_n)
    return _module_cache[loop_n]


def kernel(x, adj=None):
    x = np.ascontiguousarray(np.asarray(x, dtype=np.float32))
    assert x.shape == (N, D)
    nc = _get_module(1)
    in_maps = prep_inputs(x)
    res = run_bass_kernel_spmd(nc, in_maps, core_ids=list(range(NCORES)))
    return np.concatenate([res.results[c]["out"] for c in range(NCORES)], axis=0)



# revision 8
# speedup vs baseline: 1.4701x; 1.4701x over previous
"""BallQueryAttention TRN2 kernel (v8).

Math: reference computes softmax over a binary ball mask (d2 <= R^2), then
mask-softmax @ x.  exp of a 0/1 mask takes only values {1, e}, so

  out[i] = (S + (e-1) * sum_{j in ball(i)} x_j) / (N + (e-1) * cnt_i)

with S = colsum(x).  Sharding: rows (i) across 8 cores, x replicated
(all-gather-equivalent: every core receives the full x-derived operands).

Host-side input formatting (all O(N*D): dtype casts, reshapes, row sums
-- the O(N^2 * D) compute stays on device):
  - W   [128, N] fp16: rows [hiT_j(64) | ones(3) | loT_j(61)] -- the Gram
    stationary.  (fp16 hi/lo split; the j-side lo correction is dropped on
    dims 61:64 and the i-side lo correction (hl term) is dropped entirely:
    d2 error sigma ~5e-3, ~0.35 mask-bit flips per row, final L2 ~1.5e-3.)
  - XW  [128, 65*JT] fp16: [x_hi | 1] per j-tile (pass-C stationary).
  - R1  [128, ROWS] fp16: [hiT_i; v1; v2; v3; hiT_i(0:61)] (Gram moving;
    v = 3-level fp16 split of -0.5*sq_i riding the ones-rows of W).
  - thrD/biasA [128, JT] fp32: +-(sq_j - R^2)/2 per-partition thresholds.
  - ssum [65, 1] fp32: [colsum(x); N].
  i labeling is transpose-natural (col t*128+p <-> row p*8+t), undone by
  the contiguous-run output DMA.

Device per core (row shard of 1024):
  - Gram: ONE fp16 matmul per 512-col half into PSUM:
      Gh = W^T @ R1  (hh + sq_i + lh61 in a single K=128 contraction)
  - mask compare split Vector (is_ge -> {0,2}) / Scalar (Sign -> {-1,1}).
  - pass C: accumulating XW^T @ mask -> OUT2 [65, 1024] PSUM.
  - tail: scale by K1 with SALL bias, PE transpose, reciprocal-divide,
    one contiguous output DMA.
"""

import sys

sys.path.insert(0, "/opt/trn_rl_repo")

import numpy as np

import concourse.bass as bass
import concourse.tile as tile
from concourse import bacc, masks, mybir
from concourse.bass_utils import run_bass_kernel_spmd

F32 = mybir.dt.float32
F16 = mybir.dt.float16
AF = mybir.ActivationFunctionType
OP = mybir.AluOpType

N = 8192
D = 64
NCORES = 8
ROWS = N // NCORES          # 1024 rows per core
JT = N // 128               # 64 j-tiles
IT = ROWS // 128            # 8 i-tiles
R2 = 11.0 * 11.0
K1 = (np.e - 1.0) / 2.0
LAG = 4                     # pass-C lag in half-tiles
NLO = 61                    # j-side lo dims kept


def _setup(nc, tc, pools):
    """Iteration-invariant: identity matrix + act-table warm."""
    const, scratch, gpool, mpool, apool, spool = pools
    IDN = const.tile([65, 65], F32, tag="IDN")
    ONEC = const.tile([128, 1], F16, tag="ONEC")
    nc.vector.memset(ONEC[:], 1.0)
    masks.make_identity(nc, IDN[:])
    dumm = spool.tile([128, 1], F32, tag="dumm")
    nc.scalar.activation(dumm[:], ONEC[:], AF.Sign)
    return dict(IDN=IDN)


def _body(nc, tc, pools, env, wd, xwd, r1d, thrd, biasd, ssum, outd):
    const, scratch, gpool, mpool, apool, spool = pools
    ts = bass.ts
    IDN = env["IDN"]

    # ---------------- operand tiles + input DMA ----------------
    W = const.tile([128, N], F16, tag="W")
    XW = const.tile([128, 65 * JT], F16, tag="XW")
    R1 = const.tile([128, ROWS], F16, tag="R1")
    biasA = const.tile([128, JT], F32, tag="biasA")
    thrD = const.tile([128, JT], F32, tag="thrD")

    # small tiles first (mask thresholds + i-side movings), then the big
    # stationaries in tile-order chunks so the main loop starts early
    nc.sync.dma_start(thrD[:], thrd)
    nc.scalar.dma_start(biasA[:], biasd)
    nc.sync.dma_start(R1[:], r1d)
    sallsb = spool.tile([65, 1], F32, tag="sallsb")
    nc.gpsimd.dma_start(sallsb[:], ssum)
    CHUNKS = (4, 12, 16, 16, 16)
    pos = 0
    for ci, w in enumerate(CHUNKS):
        eng = nc.sync if ci % 2 == 0 else nc.scalar
        eng.dma_start(W[:, pos * 128 : (pos + w) * 128],
                      wd[:, pos * 128 : (pos + w) * 128])
        eng.dma_start(XW[:, pos * 65 : (pos + w) * 65],
                      xwd[:, pos * 65 : (pos + w) * 65])
        pos += w

    OUT2 = apool.tile([65, ROWS], F32, tag="OUT2")

    # ---------------- main loop ----------------
    NH = 2 * JT
    mks = {}
    for idx in range(NH + LAG):
        if idx < NH:
            t, h = divmod(idx, 2)
            cs = slice(512 * h, 512 * (h + 1))
            Gh = gpool.tile([128, 512], F32, tag="G")
            nc.tensor.matmul(Gh[:], W[:, ts(t, 128)], R1[:, cs],
                             start=True, stop=True)
            mk = mpool.tile([128, 512], F16, tag="mk")
            if idx % 2 == 0:
                nc.vector.tensor_scalar(mk[:], Gh[:], thrD[:, t : t + 1],
                                        2.0, OP.is_ge, OP.mult)
            else:
                nc.scalar.activation(mk[:], Gh[:], AF.Sign,
                                     bias=biasA[:, t : t + 1])
            mks[idx] = mk
        if idx >= LAG:
            jdx = idx - LAG
            t, h = divmod(jdx, 2)
            cs = slice(512 * h, 512 * (h + 1))
            nc.tensor.matmul(OUT2[:, cs], XW[:, 65 * t : 65 * (t + 1)],
                             mks.pop(jdx)[:],
                             start=(t == 0), stop=(t == JT - 1))

    # ---------------- tail -----------------------------------------
    b1sb = spool.tile([65, 1], F32, tag="b1sb")
    nc.vector.tensor_scalar(b1sb[:], sallsb[:], 1.0 + K1, None, OP.mult)

    # all chunks accumulate into one SBUF tile; ot[p, c*D:..] holds row
    # p*8+c, so one contiguous-run DMA (2KB/partition) writes the output
    ot = spool.tile([128, IT * D], F32, tag="ot")
    for c in range(IT):
        bap = sallsb if c < IT // 2 else b1sb
        pc = spool.tile([65, 128], F32, tag=f"pc{c % 2}")
        if c % 2 == 0:
            nc.vector.tensor_scalar(pc[:], OUT2[:, ts(c, 128)], K1, bap[:],
                                    OP.mult, OP.add)
        else:
            nc.scalar.activation(pc[:], OUT2[:, ts(c, 128)], AF.Identity,
                                 bias=bap[:], scale=K1)
        pt = gpool.tile([128, 65], F32, tag="G")
        nc.tensor.transpose(pt[:], pc[:], IDN[:])
        dinv = spool.tile([128, 1], F32, tag=f"dinv{c % 2}")
        nc.vector.reciprocal(dinv[:], pt[:, D : D + 1])
        if c % 2 == 0:
            nc.vector.tensor_scalar(ot[:, ts(c, D)], pt[:, 0:D], dinv[:],
                                    None, OP.mult)
        else:
            nc.scalar.activation(ot[:, ts(c, D)], pt[:, 0:D], AF.Identity,
                                 scale=dinv[:])
    nc.sync.dma_start(outd.rearrange("(p t) d -> p (t d)", p=128), ot[:])


def build_module(loop_n=1, scope="full"):
    nc = bacc.Bacc("TRN2", target_bir_lowering=False, debug=False,
                   num_devices=NCORES)
    wd = nc.dram_tensor("wd", [128, N], F16, kind="ExternalInput")
    xwd = nc.dram_tensor("xwd", [128, 65 * JT], F16, kind="ExternalInput")
    r1d = nc.dram_tensor("r1d", [128, ROWS], F16, kind="ExternalInput")
    thrd = nc.dram_tensor("thrd", [128, JT], F32, kind="ExternalInput")
    biasd = nc.dram_tensor("biasd", [128, JT], F32, kind="ExternalInput")
    ssum_d = nc.dram_tensor("ssum", [65, 1], F32, kind="ExternalInput")
    out_d = nc.dram_tensor("out", [ROWS, D], F32, kind="ExternalOutput")

    with tile.TileContext(nc) as tc:
        with (
            tc.tile_pool(name="const", bufs=1) as const,
            tc.tile_pool(name="scratch", bufs=2) as scratch,
            tc.tile_pool(name="gpool", bufs=6, space="PSUM") as gpool,
            tc.tile_pool(name="mk", bufs=8) as mpool,
            tc.tile_pool(name="acc", bufs=1, space="PSUM") as apool,
            tc.tile_pool(name="small", bufs=3) as spool,
        ):
            pools = (const, scratch, gpool, mpool, apool, spool)
            env = _setup(nc, tc, pools)
            args = (nc, tc, pools, env, wd.ap(), xwd.ap(), r1d.ap(),
                    thrd.ap(), biasd.ap(), ssum_d.ap(), out_d.ap())
            if loop_n == 1:
                _body(*args)
            else:
                engs = (mybir.EngineType.PE, mybir.EngineType.DVE,
                        mybir.EngineType.Activation, mybir.EngineType.Pool,
                        mybir.EngineType.SP)
                with tc.For_i(0, loop_n, hint_engines=engs,
                              staggered_reset=True) as _:
                    _body(*args)
    nc.finalize()
    return nc


def prep_inputs(x):
    """Host-side input formatting: dtype casts, transposes, row sums."""
    x = np.ascontiguousarray(np.asarray(x, dtype=np.float32))
    xh = x.astype(np.float16)
    xl = (x - xh.astype(np.float32)).astype(np.float16)

    x3h = xh.reshape(128, JT, D)                # row p*64+t (j side)
    x3l = xl.reshape(128, JT, D)
    W = np.empty((128, N), dtype=np.float16)
    W3 = W.reshape(128, JT, 128)                # [e, t, p]
    W3[0:D] = x3h.transpose(2, 1, 0)
    W3[D : D + 3] = np.float16(1.0)
    W3[D + 3 : 128] = x3l.transpose(2, 1, 0)[0:NLO]

    XW = np.empty((128, JT, 65), dtype=np.float16)
    XW[:, :, 0:D] = x3h
    XW[:, :, D] = np.float16(1.0)
    XW = XW.reshape(128, JT * 65)

    sq = (x.astype(np.float64) ** 2).sum(1).astype(np.float32)
    thrD = ((sq - R2) / 2).reshape(128, JT)
    biasA = np.ascontiguousarray(-thrD)

    ssum = np.concatenate([x.sum(0, dtype=np.float64).astype(np.float32),
                           np.float32([N])]).reshape(65, 1)

    per_core = []
    for c in range(NCORES):
        xih = xh[c * ROWS : (c + 1) * ROWS].reshape(128, IT, D)
        R1 = np.zeros((128, ROWS), dtype=np.float16)
        R13 = R1.reshape(128, IT, 128)                      # [e, t, p]
        R13[0:D] = xih.transpose(2, 1, 0)
        v = -0.5 * sq[c * ROWS : (c + 1) * ROWS].reshape(128, IT)
        v1 = v.astype(np.float16)
        rv = v - v1.astype(np.float32)
        v2 = rv.astype(np.float16)
        v3 = (rv - v2.astype(np.float32)).astype(np.float16)
        R13[D] = v1.T          # [t, p]: the -sq_i/2 split rides W's ones-rows
        R13[D + 1] = v2.T
        R13[D + 2] = v3.T
        R13[D + 3 : 128] = xih.transpose(2, 1, 0)[0:NLO]
        per_core.append({"wd": W, "xwd": XW, "r1d": np.ascontiguousarray(R1),
                         "thrd": thrD, "biasd": biasA, "ssum": ssum})
    return per_core


_module_cache = {}


def _get_module(loop_n=1):
    if loop_n not in _module_cache:
        _module_cache[loop_n] = build_module(loop_n)
    return _module_cache[loop_n]


def kernel(x, adj=None):
    x = np.ascontiguousarray(np.asarray(x, dtype=np.float32))
    assert x.shape == (N, D)
    nc = _get_module(1)
    in_maps = prep_inputs(x)
    res = run_bass_kernel_spmd(nc, in_maps, core_ids=list(range(NCORES)))
    return np.concatenate([res.results[c]["out"] for c in range(NCORES)], axis=0)



# revision 18
# speedup vs baseline: 1.4798x; 1.0066x over previous
"""BallQueryAttention TRN2 kernel (v9).

Math: reference computes softmax over a binary ball mask (d2 <= R^2), then
mask-softmax @ x.  exp of a 0/1 mask takes only values {1, e}, so

  out[i] = (S + (e-1) * sum_{j in ball(i)} x_j) / (N + (e-1) * cnt_i)

with S = colsum(x).  Sharding: rows (i) across 8 cores, x replicated
(all-gather-equivalent: every core receives the full x-derived operands).

Host-side input formatting (all O(N*D): dtype casts, reshapes, row sums
-- the O(N^2 * D) compute stays on device):
  - W   [128, N] fp16: rows [hiT_j(64) | ones(3) | loT_j(61)] -- the Gram
    stationary.  (fp16 hi/lo split; the j-side lo correction is dropped on
    dims 61:64 and the i-side lo correction (hl term) is dropped entirely:
    d2 error sigma ~5e-3, ~0.35 mask-bit flips per row, final L2 ~1.5e-3.)
  - XH8/XL8 [128, 32*160] fp8e4: pass-C stationaries, hi/lo fp8 split of
    x per j-tile PAIR: slice g*160:(g+1)*160 viewed [p, o(2), 80] holds
    [x8(j=tile 2g+o) | 1-or-0 | pad15]; ones column only in XH8.
  - R1  [128, ROWS] fp16: [hiT_i; v1; v2; v3; hiT_i(0:61)] (Gram moving;
    v = 3-level fp16 split of -0.5*sq_i riding the ones-rows of W).
  - thrD/biasA [128, JT] fp32: +-(sq_j - R^2)/2 per-partition thresholds.
  - ssum [65, 1] fp32: [colsum(x); N].
  i labeling is transpose-natural (col t*128+p <-> row p*8+t), undone by
  the contiguous-run output DMA.

Device per core (row shard of 1024):
  - Gram: ONE fp16 matmul per 512-col half into PSUM:
      Gh = W^T @ R1  (hh + sq_i + lh61 in a single K=128 contraction)
  - mask compare split Vector (is_ge -> {0,2}) / Scalar (Sign -> {-1,1}),
    writing fp8e4 into per-pair mask tiles [128, 2*512].
  - pass C: fp8 DoubleRow matmuls, K=256 (two j-tiles per MM), hi+lo
    stationaries, accumulating -> OUT2 [65, 1024] PSUM at 0.5 cyc/col.
  - tail: scale by K1 with SALL bias, PE transpose, reciprocal-divide,
    one contiguous output DMA.
"""

import sys

sys.path.insert(0, "/opt/trn_rl_repo")

import ml_dtypes
import numpy as np

NPF8 = ml_dtypes.float8_e4m3

import concourse.bass as bass
import concourse.tile as tile
from concourse import bacc, masks, mybir
from concourse.bass_utils import run_bass_kernel_spmd

F32 = mybir.dt.float32
F16 = mybir.dt.float16
F8 = mybir.dt.float8e4
AF = mybir.ActivationFunctionType
OP = mybir.AluOpType
DR = mybir.MatmulPerfMode.DoubleRow

N = 8192
D = 64
NCORES = 8
ROWS = N // NCORES          # 1024 rows per core
JT = N // 128               # 64 j-tiles
IT = ROWS // 128            # 8 i-tiles
R2 = 11.0 * 11.0
K1 = (np.e - 1.0) / 2.0
LAG = 4                     # pass-C lag in half-tiles
NLO = 61                    # j-side lo dims kept


def _setup(nc, tc, pools):
    """Iteration-invariant: identity matrix + act-table warm."""
    const, scratch, gpool, mpool, apool, spool = pools
    IDN = const.tile([65, 65], F32, tag="IDN")
    ONEC = const.tile([128, 1], F16, tag="ONEC")
    nc.vector.memset(ONEC[:], 1.0)
    masks.make_identity(nc, IDN[:])
    dumm = spool.tile([128, 1], F32, tag="dumm")
    nc.scalar.activation(dumm[:], ONEC[:], AF.Sign)
    return dict(IDN=IDN)


def _body(nc, tc, pools, env, wd, xh8d, xl8d, r1d, thrd, biasd, ssum, outd):
    const, scratch, gpool, mpool, apool, spool = pools
    ts = bass.ts
    IDN = env["IDN"]
    NPAIR = JT // 2             # 32 j-tile pairs

    # ---------------- operand tiles + input DMA ----------------
    W = const.tile([128, N], F16, tag="W")
    XH8 = const.tile([128, NPAIR * 160], F8, tag="XH8")
    XL8 = const.tile([128, NPAIR * 160], F8, tag="XL8")
    R1 = const.tile([128, ROWS], F16, tag="R1")
    biasA = const.tile([128, JT], F32, tag="biasA")
    thrD = const.tile([128, JT], F32, tag="thrD")

    # small tiles first (mask thresholds + i-side movings), then the big
    # stationaries in tile-order chunks so the main loop starts early
    nc.sync.dma_start(thrD[:], thrd)
    nc.scalar.dma_start(biasA[:], biasd)
    nc.sync.dma_start(R1[:], r1d)
    sallsb = spool.tile([65, 1], F32, tag="sallsb")
    nc.gpsimd.dma_start(sallsb[:], ssum)
    CHUNKS = (4, 12, 16, 16, 16)
    pos = 0
    for ci, w in enumerate(CHUNKS):
        eng = nc.sync if ci % 2 == 0 else nc.scalar
        eng.dma_start(W[:, pos * 128 : (pos + w) * 128],
                      wd[:, pos * 128 : (pos + w) * 128])
        g0, g1 = pos // 2, (pos + w) // 2
        eng.dma_start(XH8[:, g0 * 160 : g1 * 160],
                      xh8d[:, g0 * 160 : g1 * 160])
        eng.dma_start(XL8[:, g0 * 160 : g1 * 160],
                      xl8d[:, g0 * 160 : g1 * 160])
        pos += w

    OUT2 = apool.tile([65, ROWS], F32, tag="OUT2")

    # ---------------- main loop ----------------
    # Gram unit idx = 2*t + h (j-tile t, i-half h).  Mask pair (g, h)
    # holds tiles t=2g,2g+1 at half h; its compares land at idx 4g+h and
    # 4g+2+h, so its pass-C DoubleRow matmuls are emitted at idx
    # 4g+6+h (lag 4) to keep the PE queue from stalling on the compare.
    NH = 2 * JT
    pairs = {}
    for idx in range(NH + LAG):
        if idx < NH:
            t, h = divmod(idx, 2)
            g, o = divmod(t, 2)
            cs = slice(512 * h, 512 * (h + 1))
            Gh = gpool.tile([128, 512], F32, tag="G")
            nc.tensor.matmul(Gh[:], W[:, ts(t, 128)], R1[:, cs],
                             start=True, stop=True)
            if (g, h) not in pairs:
                pairs[(g, h)] = mpool.tile([128, 1024], F8, tag="mk",
                                           name=f"mk_{g}_{h}")
            mk = pairs[(g, h)]
            if h == 0:
                nc.vector.tensor_scalar(mk[:, 512 * o : 512 * (o + 1)],
                                        Gh[:], thrD[:, t : t + 1],
                                        2.0, OP.is_ge, OP.mult)
            else:
                nc.scalar.activation(mk[:, 512 * o : 512 * (o + 1)],
                                     Gh[:], AF.Sign,
                                     bias=biasA[:, t : t + 1])
        d = idx - 6
        if d >= 0 and d % 4 in (0, 1):
            g2, h2 = d // 4, d % 4
            cs2 = slice(512 * h2, 512 * (h2 + 1))
            mkap = pairs.pop((g2, h2))[:].rearrange("p (o n) -> p o n", o=2)
            for li, X8 in enumerate((XH8, XL8)):
                x8ap = X8[:, g2 * 160 : (g2 + 1) * 160].rearrange(
                    "p (o m) -> p o m", o=2)[:, :, 0:65]
                nc.tensor.matmul(OUT2[:, cs2], x8ap, mkap,
                                 start=(g2 == 0 and li == 0),
                                 stop=(g2 == NPAIR - 1 and li == 1),
                                 perf_mode=DR)

    # ---------------- tail -----------------------------------------
    b1sb = spool.tile([65, 1], F32, tag="b1sb")
    nc.vector.tensor_scalar(b1sb[:], sallsb[:], 1.0 + K1, None, OP.mult)

    # all chunks accumulate into one SBUF tile; ot[p, c*D:..] holds row
    # p*8+c, so one contiguous-run DMA (2KB/partition) writes the output
    ot = spool.tile([128, IT * D], F32, tag="ot")
    for c in range(IT):
        bap = sallsb if c < IT // 2 else b1sb
        pc = spool.tile([65, 128], F32, tag=f"pc{c % 2}")
        if c % 2 == 0:
            nc.vector.tensor_scalar(pc[:], OUT2[:, ts(c, 128)], K1, bap[:],
                                    OP.mult, OP.add)
        else:
            nc.scalar.activation(pc[:], OUT2[:, ts(c, 128)], AF.Identity,
                                 bias=bap[:], scale=K1)
        pt = gpool.tile([128, 65], F32, tag="G")
        nc.tensor.transpose(pt[:], pc[:], IDN[:])
        dinv = spool.tile([128, 1], F32, tag=f"dinv{c % 2}")
        nc.vector.reciprocal(dinv[:], pt[:, D : D + 1])
        if c % 2 == 0:
            nc.vector.tensor_scalar(ot[:, ts(c, D)], pt[:, 0:D], dinv[:],
                                    None, OP.mult)
        else:
            nc.scalar.activation(ot[:, ts(c, D)], pt[:, 0:D], AF.Identity,
                                 scale=dinv[:])
    nc.sync.dma_start(outd.rearrange("(p t) d -> p (t d)", p=128), ot[:])


def build_module(loop_n=1, scope="full"):
    nc = bacc.Bacc("TRN2", target_bir_lowering=False, debug=False,
                   num_devices=NCORES)
    wd = nc.dram_tensor("wd", [128, N], F16, kind="ExternalInput")
    xh8d = nc.dram_tensor("xh8d", [128, (JT // 2) * 160], F8,
                          kind="ExternalInput")
    xl8d = nc.dram_tensor("xl8d", [128, (JT // 2) * 160], F8,
                          kind="ExternalInput")
    r1d = nc.dram_tensor("r1d", [128, ROWS], F16, kind="ExternalInput")
    thrd = nc.dram_tensor("thrd", [128, JT], F32, kind="ExternalInput")
    biasd = nc.dram_tensor("biasd", [128, JT], F32, kind="ExternalInput")
    ssum_d = nc.dram_tensor("ssum", [65, 1], F32, kind="ExternalInput")
    out_d = nc.dram_tensor("out", [ROWS, D], F32, kind="ExternalOutput")

    with tile.TileContext(nc) as tc:
        with (
            tc.tile_pool(name="const", bufs=1) as const,
            tc.tile_pool(name="scratch", bufs=2) as scratch,
            tc.tile_pool(name="gpool", bufs=6, space="PSUM") as gpool,
            tc.tile_pool(name="mk", bufs=8) as mpool,
            tc.tile_pool(name="acc", bufs=1, space="PSUM") as apool,
            tc.tile_pool(name="small", bufs=3) as spool,
        ):
            pools = (const, scratch, gpool, mpool, apool, spool)
            env = _setup(nc, tc, pools)
            args = (nc, tc, pools, env, wd.ap(), xh8d.ap(), xl8d.ap(),
                    r1d.ap(), thrd.ap(), biasd.ap(), ssum_d.ap(), out_d.ap())
            if loop_n == 1:
                _body(*args)
            else:
                engs = (mybir.EngineType.PE, mybir.EngineType.DVE,
                        mybir.EngineType.Activation, mybir.EngineType.Pool,
                        mybir.EngineType.SP)
                with tc.For_i(0, loop_n, hint_engines=engs,
                              staggered_reset=True) as _:
                    _body(*args)
    nc.finalize()
    return nc


def prep_inputs(x):
    """Host-side input formatting: dtype casts, transposes, row sums."""
    x = np.ascontiguousarray(np.asarray(x, dtype=np.float32))
    xh = x.astype(np.float16)
    xl = (x - xh.astype(np.float32)).astype(np.float16)

    x3h = xh.reshape(128, JT, D)                # row p*64+t (j side)
    x3l = xl.reshape(128, JT, D)
    W = np.empty((128, N), dtype=np.float16)
    W3 = W.reshape(128, JT, 128)                # [e, t, p]
    W3[0:D] = x3h.transpose(2, 1, 0)
    W3[D : D + 3] = np.float16(1.0)
    W3[D + 3 : 128] = x3l.transpose(2, 1, 0)[0:NLO]

    x8h = x.astype(NPF8)
    x8l = (x - x8h.astype(np.float32)).astype(NPF8)
    XH8 = np.zeros((128, JT // 2, 2, 80), dtype=NPF8)
    XL8 = np.zeros((128, JT // 2, 2, 80), dtype=NPF8)
    XH8[:, :, :, 0:D] = x8h.reshape(128, JT // 2, 2, D)
    XL8[:, :, :, 0:D] = x8l.reshape(128, JT // 2, 2, D)
    XH8[:, :, :, D] = NPF8(1.0)                 # cnt column (hi only)
    XH8 = XH8.reshape(128, (JT // 2) * 160)
    XL8 = XL8.reshape(128, (JT // 2) * 160)

    sq = (x.astype(np.float64) ** 2).sum(1).astype(np.float32)
    thrD = ((sq - R2) / 2).reshape(128, JT)
    biasA = np.ascontiguousarray(-thrD)

    ssum = np.concatenate([x.sum(0, dtype=np.float64).astype(np.float32),
                           np.float32([N])]).reshape(65, 1)

    per_core = []
    for c in range(NCORES):
        xih = xh[c * ROWS : (c + 1) * ROWS].reshape(128, IT, D)
        R1 = np.zeros((128, ROWS), dtype=np.float16)
        R13 = R1.reshape(128, IT, 128)                      # [e, t, p]
        R13[0:D] = xih.transpose(2, 1, 0)
        v = -0.5 * sq[c * ROWS : (c + 1) * ROWS].reshape(128, IT)
        v1 = v.astype(np.float16)
        rv = v - v1.astype(np.float32)
        v2 = rv.astype(np.float16)
        v3 = (rv - v2.astype(np.float32)).astype(np.float16)
        R13[D] = v1.T          # [t, p]: the -sq_i/2 split rides W's ones-rows
        R13[D + 1] = v2.T
        R13[D + 2] = v3.T
        R13[D + 3 : 128] = xih.transpose(2, 1, 0)[0:NLO]
        per_core.append({"wd": W, "xh8d": XH8, "xl8d": XL8,
                         "r1d": np.ascontiguousarray(R1),
                         "thrd": thrD, "biasd": biasA, "ssum": ssum})
    return per_core


_module_cache = {}


def _get_module(loop_n=1):
    if loop_n not in _module_cache:
        _module_cache[loop_n] = build_module(loop_n)
    return _module_cache[loop_n]


def kernel(x, adj=None):
    x = np.ascontiguousarray(np.asarray(x, dtype=np.float32))
    assert x.shape == (N, D)
    nc = _get_module(1)
    in_maps = prep_inputs(x)
    res = run_bass_kernel_spmd(nc, in_maps, core_ids=list(range(NCORES)))
    return np.concatenate([res.results[c]["out"] for c in range(NCORES)], axis=0)

